# revision 24
# baseline (speedup 1.0000x reference)
"""BERT (12-layer, C=768, B=4, T=1024, V=30522) forward pass on 8 Trainium2 cores.

Sharding: sequence-parallel over the 4096 tokens (512 tokens/core; core c owns
batch item c//2, sequence half c%2). Attention K/V for the other half of the
sequence is obtained with a pair-wise AllReduce in bf16 (K_partner = K_sum -
K_mine). The LM head is token-local: each core computes its own 512 tokens
against the full (padded) vocab, so no final AllGather is needed.

Matmuls run in bf16 (fp32 PSUM accumulation). The residual stream is fp32
(stored as fp32r so the LN statistic matmuls stream at full PE rate). LN
statistics and softmax normalization are fp32. rstd = exp(-0.5*ln(var+eps))
so LN shares the ACT table set with attention's exp (no table thrash).
Partition broadcasts (softmax 1/denom, LN stats) run on GpSimd, keeping the
tensor engine stream free of scalar-chain stalls.
"""
import math
import sys
from contextlib import ExitStack

for _p in ("/opt/trn_rl_repo",):
    if _p not in sys.path:
        sys.path.insert(0, _p)

import numpy as np

import concourse.bass as bass
import concourse.mybir as mybir
import concourse.tile as tile
from concourse.tile_rust import add_dep_helper
from concourse import bacc
from concourse.bass_utils import run_bass_kernel_spmd
from concourse.masks import make_identity

NC = 8          # cores
L = 12          # layers
C = 768         # model dim
H = 12          # heads
D = 64          # head dim
V = 30522       # vocab
B, T = 4, 1024
TQ = 512        # tokens per core
KC = C // 128   # 6 c-chunks
VPAD = 30720    # vocab padded to 60*512
NVB = VPAD // 512   # 60 vocab blocks
LN_EPS = 1e-5

F32 = mybir.dt.float32
F32R = mybir.dt.float32r
BF16 = mybir.dt.bfloat16
AF = mybir.ActivationFunctionType
ALU = mybir.AluOpType
PAIRS = [[0, 1], [2, 3], [4, 5], [6, 7]]


def _layernorm(nc, scr, tiny, pbc, psp, x_tiles, out_pool, out_tag,
               onescr, onescb, affine, lnw_ap, lnb_ap):
    """LN over the feature (partition) axis of feature-major fp32r x tiles.

    Returns KC bf16 output tiles [128, TQ]. Stats fp32; rstd via
    exp(-0.5*ln(var+eps)) (shares the exp ACT table set). Stat broadcast on
    GpSimd (no PE involvement beyond the two reduction matmul chains).
    """
    sq = []
    for c in range(KC):
        sqt = scr.tile([128, TQ], BF16, tag="sq", name=f"sq{c}")
        nc.vector.tensor_mul(sqt[:], x_tiles[c][:].bitcast(F32),
                             x_tiles[c][:].bitcast(F32))
        sq.append(sqt)
    s1 = psp.tile([128, TQ], F32, tag="ps")
    for c in range(KC):
        nc.tensor.matmul(s1[:1, :], onescr[:], x_tiles[c][:], start=(c == 0),
                         stop=(c == KC - 1))
    s2 = psp.tile([128, TQ], F32, tag="ps")
    for c in range(KC):
        nc.tensor.matmul(s2[:1, :], onescb[:], sq[c][:], start=(c == 0),
                         stop=(c == KC - 1))
    # f32 scratch slots: 0 = m, 1 = e2+eps then msd, 2 = m^2 then ln, 3 = rstd
    lnt = tiny.tile([1, 4 * TQ], F32, tag="lnt", bufs=2)

    def sl(i):
        return lnt[0:1, i * TQ:(i + 1) * TQ]

    inv = 1.0 / C
    nc.vector.tensor_scalar_mul(sl(0), s1[:1, :], inv)              # m
    nc.vector.tensor_scalar(sl(1), s2[:1, :], inv, LN_EPS,
                            ALU.mult, ALU.add)                      # E[x^2]+eps
    nc.vector.tensor_mul(sl(2), sl(0), sl(0))                       # m^2
    nc.vector.tensor_sub(sl(1), sl(1), sl(2))                       # var+eps
    nc.scalar.activation(sl(2), sl(1), AF.Sqrt)                     # sd
    nc.vector.reciprocal(sl(3), sl(2))                              # rstd
    nc.vector.tensor_mul(sl(1), sl(0), sl(3))                       # m*rstd
    bc = pbc.tile([128, 2 * TQ], F32, tag="bc", bufs=2)
    nc.gpsimd.partition_broadcast(bc[:, 0:TQ], sl(3))               # rstd bcast
    nc.gpsimd.partition_broadcast(bc[:, TQ:2 * TQ], sl(1))          # msd bcast
    out = []
    for c in range(KC):
        t1 = scr.tile([128, TQ], F32, tag="lnt1")
        nc.vector.tensor_mul(t1[:], x_tiles[c][:].bitcast(F32), bc[:, 0:TQ])
        o = out_pool.tile([128, TQ], BF16, tag=out_tag)
        if affine:
            t2 = scr.tile([128, TQ], F32, tag="lnt1")
            nc.vector.tensor_sub(t2[:], t1[:], bc[:, TQ:2 * TQ])
            nc.vector.tensor_scalar(o[:], t2[:], lnw_ap[:, c:c + 1],
                                    lnb_ap[:, c:c + 1], ALU.mult, ALU.add)
        else:
            nc.vector.tensor_sub(o[:], t1[:], bc[:, TQ:2 * TQ])
        out.append(o)
    return out


def build(flags: tuple, n_layers: int = L, do_lm: bool = True) -> bacc.Bacc:
    ln_affine, attn_bias, proj_bias, fc_bias, mproj_bias = flags
    nc = bacc.Bacc("TRN2", target_bir_lowering=False, num_devices=NC)

    # ---- external IO ----
    idx = nc.dram_tensor("idx", [TQ, 1], mybir.dt.int32, kind="ExternalInput")
    wte = nc.dram_tensor("wte", [V, C], BF16, kind="ExternalInput")
    wpe = nc.dram_tensor("wpe", [TQ, C], BF16, kind="ExternalInput")
    aw = nc.dram_tensor("aw", [L, C, 3 * C], BF16, kind="ExternalInput")
    pw = nc.dram_tensor("pw", [L, C, C], BF16, kind="ExternalInput")
    fw = nc.dram_tensor("fw", [L, C, 4 * C], BF16, kind="ExternalInput")
    mw = nc.dram_tensor("mw", [L, 4 * C, C], BF16, kind="ExternalInput")
    lmw = nc.dram_tensor("lmw", [NVB, 128, KC * 512], BF16, kind="ExternalInput")
    onesr_d = nc.dram_tensor("onesr", [1, 128], BF16, kind="ExternalInput")
    onescr_d = nc.dram_tensor("onescr", [128, 1], F32R, kind="ExternalInput")
    onescb_d = nc.dram_tensor("onescb", [128, 1], BF16, kind="ExternalInput")
    vones_d = nc.dram_tensor("vones", [128, H], BF16, kind="ExternalInput")
    if ln_affine:
        lnw_d = nc.dram_tensor("lnw", [2 * L + 1, 128, KC], F32, kind="ExternalInput")
        lnb_d = nc.dram_tensor("lnb", [2 * L + 1, 128, KC], F32, kind="ExternalInput")
    if attn_bias:
        abpp_d = nc.dram_tensor("abpp", [L, 128, 12], F32, kind="ExternalInput")
        abrow_d = nc.dram_tensor("abrow", [L, 1, 3 * C], BF16, kind="ExternalInput")
    if proj_bias:
        pb_d = nc.dram_tensor("pb", [L, 128, KC], F32, kind="ExternalInput")
    if fc_bias:
        fcb_d = nc.dram_tensor("fcb", [L, 128, 4 * KC], F32, kind="ExternalInput")
    if mproj_bias:
        mb_d = nc.dram_tensor("mb", [L, 128, KC], F32, kind="ExternalInput")
    logits = nc.dram_tensor("logits", [TQ, VPAD], F32, kind="ExternalOutput")

    with tile.TileContext(nc) as tc, ExitStack() as stack:
        cst = stack.enter_context(tc.tile_pool(name="cst", bufs=1))
        px = stack.enter_context(tc.tile_pool(name="px", bufs=6))
        tiny = stack.enter_context(tc.tile_pool(name="tiny", bufs=3))
        pbc = stack.enter_context(tc.tile_pool(name="pbc", bufs=2))
        dram = stack.enter_context(tc.tile_pool(name="dram", bufs=2, space="DRAM"))
        # persistent weight pool -> deep cross-phase prefetch
        pw768 = stack.enter_context(tc.tile_pool(name="pw768", bufs=20))
        psp = stack.enter_context(tc.tile_pool(name="psp", bufs=8, space="PSUM"))

        onesr = cst.tile([1, 128], BF16, tag="onesr")
        nc.sync.dma_start(onesr[:], onesr_d[:])
        onescr = cst.tile([128, 1], F32R, tag="onescr")
        nc.sync.dma_start(onescr[:], onescr_d[:])
        onescb = cst.tile([128, 1], BF16, tag="onescb")
        nc.sync.dma_start(onescb[:], onescb_d[:])
        vones = cst.tile([128, H], BF16, tag="vones")
        nc.sync.dma_start(vones[:], vones_d[:])
        ident = cst.tile([128, 128], BF16, tag="ident")
        make_identity(nc, ident[:])
        if ln_affine:
            lnw_sb = cst.tile([128, (2 * L + 1) * KC], F32, tag="lnw")
            nc.sync.dma_start(lnw_sb[:], lnw_d[:].rearrange("a p c -> p (a c)"))
            lnb_sb = cst.tile([128, (2 * L + 1) * KC], F32, tag="lnb")
            nc.sync.dma_start(lnb_sb[:], lnb_d[:].rearrange("a p c -> p (a c)"))
        if attn_bias:
            abpp_sb = cst.tile([128, L * 12], F32, tag="abpp")
            nc.sync.dma_start(abpp_sb[:], abpp_d[:].rearrange("a p c -> p (a c)"))
            abrow_sb = cst.tile([1, L * 3 * C], BF16, tag="abrow")
            nc.sync.dma_start(abrow_sb[:], abrow_d[:].rearrange("a p c -> p (a c)"))
        if proj_bias:
            pb_sb = cst.tile([128, L * KC], F32, tag="pb")
            nc.sync.dma_start(pb_sb[:], pb_d[:].rearrange("a p c -> p (a c)"))
        if fc_bias:
            fcb_sb = cst.tile([128, L * 4 * KC], F32, tag="fcb")
            nc.sync.dma_start(fcb_sb[:], fcb_d[:].rearrange("a p c -> p (a c)"))
        if mproj_bias:
            mb_sb = cst.tile([128, L * KC], F32, tag="mb")
            nc.sync.dma_start(mb_sb[:], mb_d[:].rearrange("a p c -> p (a c)"))

        # residual stream, fp32r, persistent
        x_tiles = [px.tile([128, TQ], F32R, tag="x", name=f"x{i}")
                   for i in range(KC)]

        # ---- embedding: x = wte[idx] + wpe ----
        with tc.tile_pool(name="emb", bufs=5) as emb:
            for tt in range(4):
                it = emb.tile([128, 1], mybir.dt.int32, tag="it")
                nc.sync.dma_start(it[:], idx[128 * tt:128 * (tt + 1), :])
                g = emb.tile([128, C], BF16, tag="g")
                nc.gpsimd.indirect_dma_start(
                    out=g[:], out_offset=None, in_=wte[:],
                    in_offset=bass.IndirectOffsetOnAxis(ap=it[:, :1], axis=0))
                wp = emb.tile([128, C], BF16, tag="wp")
                nc.sync.dma_start(wp[:], wpe[128 * tt:128 * (tt + 1), :])
                xa = emb.tile([128, C], BF16, tag="xa")
                nc.vector.tensor_add(xa[:], g[:], wp[:])
                for cc in range(KC):
                    pt = psp.tile([128, 128], BF16, tag="ps")
                    nc.tensor.transpose(pt[:], xa[:, 128 * cc:128 * (cc + 1)],
                                        ident[:])
                    nc.vector.tensor_copy(
                        x_tiles[cc][:, 128 * tt:128 * (tt + 1)], pt[:])

        # ---- transformer layers ----
        for l in range(n_layers):
            # ===== Scope A: LN1, QKV, KV exchange, attention, proj =====
            with tc.tile_pool(name="sa_scr", bufs=8) as scr, \
                 tc.tile_pool(name="sa_h", bufs=6) as ph, \
                 tc.tile_pool(name="sa_qk", bufs=12) as pqk, \
                 tc.tile_pool(name="sa_kr", bufs=6) as pkr, \
                 tc.tile_pool(name="sa_v", bufs=8) as pv, \
                 tc.tile_pool(name="sa_at", bufs=6) as pat, \
                 tc.tile_pool(name="sa_rs", bufs=4) as prs, \
                 tc.tile_pool(name="sa_yl", bufs=12) as pyl, \
                 tc.tile_pool(name="sa_y", bufs=6) as py:

                h = _layernorm(
                    nc, scr, tiny, pbc, psp, x_tiles, ph, "h", onescr, onescb,
                    ln_affine,
                    lnw_sb[:, 2 * l * KC:(2 * l + 1) * KC] if ln_affine else None,
                    lnb_sb[:, 2 * l * KC:(2 * l + 1) * KC] if ln_affine else None)

                k_loc = dram.tile([C, TQ], BF16, tag="kloc")
                k_sum = dram.tile([C, TQ], BF16, tag="ksum")
                v_loc = dram.tile([TQ, H * (D + 1)], BF16, tag="vloc")
                v_sum = dram.tile([TQ, H * (D + 1)], BF16, tag="vsum")

                # --- V (token-major, exchange kicked first: its AllReduce
                # --- result is needed latest into the remote pass) ---
                wv = []
                for kc in range(KC):
                    wt = pw768.tile([128, C], BF16, tag="w768", name=f"wv{kc}")
                    nc.sync.dma_start(wt[:], aw[l, 128 * kc:128 * (kc + 1),
                                                2 * C:3 * C])
                    wv.append(wt)
                v_t = []
                for tt in range(4):
                    va = psp.tile([128, TQ], F32, tag="ps")
                    vb2 = psp.tile([128, TQ], F32, tag="ps")
                    first = 0
                    if attn_bias:
                        brow = abrow_sb[:, l * 3 * C + 2 * C:l * 3 * C + 3 * C]
                        nc.tensor.matmul(va[:, :512], onesr[:, :128],
                                         brow[:, 0:512], start=True, stop=False)
                        nc.tensor.matmul(vb2[:, :256], onesr[:, :128],
                                         brow[:, 512:768], start=True,
                                         stop=False)
                        first = 1
                    for kc in range(KC):
                        lhs = h[kc][:, 128 * tt:128 * (tt + 1)]
                        nc.tensor.matmul(va[:, :512], lhs, wv[kc][:, 0:512],
                                         start=(kc == 0 and not first),
                                         stop=(kc == KC - 1))
                        nc.tensor.matmul(vb2[:, :256], lhs, wv[kc][:, 512:768],
                                         start=(kc == 0 and not first),
                                         stop=(kc == KC - 1))
                    vt = pv.tile([128, H * (D + 1)], BF16, tag="v",
                                 name=f"vt{tt}")
                    vv = vt[:].rearrange("p (h e) -> p h e", e=D + 1)
                    nc.vector.tensor_copy(
                        vv[:, 0:8, 0:D],
                        va[:, :512].rearrange("p (h e) -> p h e", e=D))
                    nc.vector.tensor_copy(
                        vv[:, 8:12, 0:D],
                        vb2[:, :256].rearrange("p (h e) -> p h e", e=D))
                    nc.vector.tensor_copy(
                        vv[:, :, D:D + 1],
                        vones[:].rearrange("p (h o) -> p h o", o=1))
                    v_t.append(vt)
                    nc.sync.dma_start(v_loc[128 * tt:128 * (tt + 1), :], vt[:])
                nc.gpsimd.collective_compute(
                    "AllReduce", ALU.add, replica_groups=PAIRS,
                    ins=[v_loc.opt()], outs=[v_sum.opt()])

                # --- K (exchange kicked second) ---
                wk = []
                for kc in range(KC):
                    wt = pw768.tile([128, C], BF16, tag="w768", name=f"wk{kc}")
                    nc.sync.dma_start(wt[:], aw[l, 128 * kc:128 * (kc + 1),
                                                C:2 * C])
                    wk.append(wt)
                k_t = []
                for oc in range(KC):
                    psl = psp.tile([128, TQ], F32, tag="ps")
                    for kc in range(KC):
                        nc.tensor.matmul(
                            psl[:], wk[kc][:, 128 * oc:128 * (oc + 1)],
                            h[kc][:], start=(kc == 0), stop=(kc == KC - 1))
                    dst = pqk.tile([128, TQ], BF16, tag="qk", name=f"kt{oc}")
                    if attn_bias:
                        nc.vector.tensor_scalar_add(
                            dst[:], psl[:],
                            abpp_sb[:, l * 12 + KC + oc:l * 12 + KC + oc + 1])
                    else:
                        nc.scalar.copy(dst[:], psl[:])
                    k_t.append(dst)
                    nc.sync.dma_start(k_loc[128 * oc:128 * (oc + 1), :], dst[:])
                nc.gpsimd.collective_compute(
                    "AllReduce", ALU.add, replica_groups=PAIRS,
                    ins=[k_loc.opt()], outs=[k_sum.opt()])

                # --- Q ---
                wq = []
                for kc in range(KC):
                    wt = pw768.tile([128, C], BF16, tag="w768", name=f"wq{kc}")
                    nc.sync.dma_start(wt[:], aw[l, 128 * kc:128 * (kc + 1),
                                                0:C])
                    wq.append(wt)
                q_t = []
                for oc in range(KC):
                    psl = psp.tile([128, TQ], F32, tag="ps")
                    for kc in range(KC):
                        nc.tensor.matmul(
                            psl[:], wq[kc][:, 128 * oc:128 * (oc + 1)],
                            h[kc][:], start=(kc == 0), stop=(kc == KC - 1))
                    dst = pqk.tile([128, TQ], BF16, tag="qk", name=f"qt{oc}")
                    if attn_bias:
                        nc.vector.tensor_scalar_add(
                            dst[:], psl[:],
                            abpp_sb[:, l * 12 + oc:l * 12 + oc + 1])
                    else:
                        nc.scalar.copy(dst[:], psl[:])
                    q_t.append(dst)

                # --- remote K/V: load pair sum, subtract mine (in place) ---
                # prefetch proj weights BEFORE the k_sum/v_sum loads: those
                # loads wait on the AllReduce semaphore and block every DMA
                # issued after them on the sync queue
                wp_ = []
                for kc in range(KC):
                    wt = pw768.tile([128, C], BF16, tag="w768", name=f"wp{kc}")
                    nc.sync.dma_start(wt[:], pw[l, 128 * kc:128 * (kc + 1), :])
                    wp_.append(wt)

                # remote K/V: load pair sum (V first — its exchange finishes
                # first), subtract mine (in place)
                v_r = []
                for tt in range(4):
                    vst = pv.tile([128, H * (D + 1)], BF16, tag="v",
                                  name=f"vs{tt}")
                    nc.sync.dma_start(vst[:],
                                      v_sum[128 * tt:128 * (tt + 1), :])
                    nc.vector.tensor_sub(vst[:], vst[:], v_t[tt][:])
                    v_r.append(vst)
                k_r = []
                for kc in range(KC):
                    kst = pkr.tile([128, TQ], BF16, tag="kr", name=f"ks{kc}")
                    nc.sync.dma_start(kst[:],
                                      k_sum[128 * kc:128 * (kc + 1), :])
                    nc.vector.tensor_sub(kst[:], kst[:], k_t[kc][:])
                    k_r.append(kst)

                # --- attention pass 1: local halves of all 12 heads; each
                # --- head's partial (AV rows + denom row) is evacuated to
                # --- SBUF so the PE never has to wait for the AllReduce
                y_t = [py.tile([128, TQ], BF16, tag="y", name=f"y{i}")
                       for i in range(KC)]
                yloc = []
                scale = 1.0 / math.sqrt(D)
                for hh in range(H):
                    ct, ro = hh // 2, 64 * (hh % 2)
                    yp = psp.tile([128, TQ], F32, tag="ps", name=f"ypl{hh}")
                    for sc in range(4):
                        ss = psp.tile([128, TQ], F32, tag="ps")
                        nc.tensor.matmul(
                            ss[:], k_t[ct][ro:ro + D, 128 * sc:128 * (sc + 1)],
                            q_t[ct][ro:ro + D, :], start=True, stop=True)
                        at = pat.tile([128, TQ], BF16, tag="at")
                        nc.scalar.activation(at[:], ss[:], AF.Exp, scale=scale)
                        nc.tensor.matmul(
                            yp[:D + 1, :],
                            v_t[sc][:, hh * (D + 1):(hh + 1) * (D + 1)],
                            at[:], start=(sc == 0), stop=(sc == 3))
                    yl = pyl.tile([D + 1, TQ], BF16, tag="yl", name=f"yl{hh}")
                    nc.vector.tensor_copy(yl[:], yp[:D + 1, :])
                    yloc.append(yl)

                # --- attention pass 2: remote halves + softmax normalize ---
                for hh in range(H):
                    ct, ro = hh // 2, 64 * (hh % 2)
                    yp = psp.tile([128, TQ], F32, tag="ps", name=f"ypr{hh}")
                    for sc in range(4):
                        ss = psp.tile([128, TQ], F32, tag="ps")
                        nc.tensor.matmul(
                            ss[:], k_r[ct][ro:ro + D, 128 * sc:128 * (sc + 1)],
                            q_t[ct][ro:ro + D, :], start=True, stop=True)
                        at = pat.tile([128, TQ], BF16, tag="at")
                        nc.scalar.activation(at[:], ss[:], AF.Exp, scale=scale)
                        nc.tensor.matmul(
                            yp[:D + 1, :],
                            v_r[sc][:, hh * (D + 1):(hh + 1) * (D + 1)],
                            at[:], start=(sc == 0), stop=(sc == 3))
                    rec = tiny.tile([1, 2 * TQ], F32, tag="rec", bufs=4)
                    nc.vector.tensor_add(rec[0:1, 0:TQ], yloc[hh][D:D + 1, :],
                                         yp[D:D + 1, :])
                    nc.vector.reciprocal(rec[0:1, TQ:2 * TQ], rec[0:1, 0:TQ])
                    rsb = prs.tile([64, TQ], F32, tag="rsb")
                    nc.gpsimd.partition_broadcast(rsb[:], rec[0:1, TQ:2 * TQ])
                    tsum = scr.tile([64, TQ], F32, tag="tsum", bufs=3)
                    nc.vector.tensor_add(tsum[:], yp[:D, :], yloc[hh][:D, :])
                    nc.vector.tensor_mul(y_t[ct][ro:ro + D, :], tsum[:],
                                         rsb[:])

                # --- proj + residual ---
                for oc in range(KC):
                    psl = psp.tile([128, TQ], F32, tag="ps")
                    for kc in range(KC):
                        nc.tensor.matmul(
                            psl[:], wp_[kc][:, 128 * oc:128 * (oc + 1)],
                            y_t[kc][:], start=(kc == 0), stop=(kc == KC - 1))
                    if proj_bias:
                        nc.vector.scalar_tensor_tensor(
                            x_tiles[oc][:], psl[:],
                            pb_sb[:, l * KC + oc:l * KC + oc + 1],
                            x_tiles[oc][:].bitcast(F32), ALU.add, ALU.add)
                    else:
                        nc.vector.tensor_add(x_tiles[oc][:],
                                             x_tiles[oc][:].bitcast(F32),
                                             psl[:])

            # ===== Scope B: LN2, fc+gelu, mproj =====
            with tc.tile_pool(name="sb_scr", bufs=8) as scr, \
                 tc.tile_pool(name="sb_h", bufs=6) as ph, \
                 tc.tile_pool(name="sb_g", bufs=25) as pg:

                h2 = _layernorm(
                    nc, scr, tiny, pbc, psp, x_tiles, ph, "h", onescr, onescb,
                    ln_affine,
                    lnw_sb[:, (2 * l + 1) * KC:(2 * l + 2) * KC] if ln_affine else None,
                    lnb_sb[:, (2 * l + 1) * KC:(2 * l + 2) * KC] if ln_affine else None)

                g_t = []
                for ob in range(4):
                    wf = []
                    for kc in range(KC):
                        wt = pw768.tile([128, C], BF16, tag="w768",
                                        name=f"wf{kc}")
                        nc.sync.dma_start(
                            wt[:], fw[l, 128 * kc:128 * (kc + 1),
                                      C * ob:C * (ob + 1)])
                        wf.append(wt)
                    for oc in range(KC):
                        psl = psp.tile([128, TQ], F32, tag="ps")
                        for kc in range(KC):
                            nc.tensor.matmul(
                                psl[:], wf[kc][:, 128 * oc:128 * (oc + 1)],
                                h2[kc][:], start=(kc == 0), stop=(kc == KC - 1))
                        gt = pg.tile([128, TQ], BF16, tag="g")
                        ob_oc = ob * KC + oc
                        bias = (fcb_sb[:, l * 4 * KC + ob_oc:
                                       l * 4 * KC + ob_oc + 1]
                                if fc_bias else 0.0)
                        last_gelu = nc.scalar.activation(
                            gt[:], psl[:], AF.Gelu_apprx_tanh, bias=bias)
                        g_t.append(gt)

                # mproj: kc-outer so g tiles stream; 6 accumulators live
                # (4 from "ps" + 2 parked in the idle "ss" slots)
                psl6 = [psp.tile([128, TQ], F32, tag="ps", name=f"m{i}")[:]
                        for i in range(KC)]
                for k in range(4 * KC):
                    wt = pw768.tile([128, C], BF16, tag="w768", name="wm")
                    nc.sync.dma_start(wt[:], mw[l, 128 * k:128 * (k + 1), :])
                    for oc in range(KC):
                        nc.tensor.matmul(
                            psl6[oc][:], wt[:, 128 * oc:128 * (oc + 1)],
                            g_t[k][:], start=(k == 0), stop=(k == 4 * KC - 1))
                for oc in range(KC):
                    if mproj_bias:
                        nc.vector.scalar_tensor_tensor(
                            x_tiles[oc][:], psl6[oc][:],
                            mb_sb[:, l * KC + oc:l * KC + oc + 1],
                            x_tiles[oc][:].bitcast(F32), ALU.add, ALU.add)
                    else:
                        nc.vector.tensor_add(x_tiles[oc][:],
                                             x_tiles[oc][:].bitcast(F32),
                                             psl6[oc][:])

        # ---- final LN + token-local LM head over the full padded vocab ----
        if do_lm:
            with tc.tile_pool(name="fl_scr", bufs=8) as scr, \
                 tc.tile_pool(name="fl_h", bufs=6) as ph, \
                 tc.tile_pool(name="lm_w", bufs=4) as plw, \
                 tc.tile_pool(name="lm_out", bufs=4) as plo:
                xf = _layernorm(
                    nc, scr, tiny, pbc, psp, x_tiles, ph, "h", onescr, onescb,
                    ln_affine,
                    lnw_sb[:, 2 * L * KC:(2 * L + 1) * KC] if ln_affine else None,
                    lnb_sb[:, 2 * L * KC:(2 * L + 1) * KC] if ln_affine else None)
                for vb in range(NVB):
                    lw = plw.tile([128, KC * 512], BF16, tag="lw")
                    nc.sync.dma_start(lw[:], lmw[vb])
                    for m in range(4):
                        psl = psp.tile([128, TQ], F32, tag="ps")
                        for kc in range(KC):
                            nc.tensor.matmul(
                                psl[:], xf[kc][:, 128 * m:128 * (m + 1)],
                                lw[:, 512 * kc:512 * (kc + 1)],
                                start=(kc == 0), stop=(kc == KC - 1))
                        osb = plo.tile([128, TQ], F32, tag="lo")
                        if m % 2 == 0:
                            nc.scalar.copy(osb[:], psl[:])
                        else:
                            nc.vector.tensor_copy(osb[:], psl[:])
                        nc.sync.dma_start(
                            logits[128 * m:128 * (m + 1),
                                   512 * vb:512 * (vb + 1)], osb[:])

    nc.compile()
    return nc


_CACHE = {}


def _get_nc(flags):
    if flags not in _CACHE:
        _CACHE[flags] = build(flags)
    return _CACHE[flags]


def _bf16(x):
    import ml_dtypes
    return np.asarray(x, dtype=np.float32).astype(ml_dtypes.bfloat16)


def kernel(idx, wte, wpe, ln1_w, ln1_b, attn_w, attn_b, proj_w, proj_b,
           ln2_w, ln2_b, fc_w, fc_b, mproj_w, mproj_b, lnf_w, lnf_b, lm_head_w):
    idx = np.asarray(idx)
    idx_flat = idx.reshape(B * T).astype(np.int32)
    wte_b = _bf16(wte)
    wpe_b = _bf16(np.asarray(wpe, np.float32)[:T])

    ln_affine = not (
        np.all(ln1_w == 1) and np.all(ln1_b == 0) and np.all(ln2_w == 1)
        and np.all(ln2_b == 0) and np.all(lnf_w == 1) and np.all(lnf_b == 0))
    attn_bias = bool(np.any(attn_b != 0))
    proj_bias = bool(np.any(proj_b != 0))
    fc_bias = bool(np.any(fc_b != 0))
    mproj_bias = bool(np.any(mproj_b != 0))
    flags = (ln_affine, attn_bias, proj_bias, fc_bias, mproj_bias)
    nc = _get_nc(flags)

    # host-side layout prep: transpose weights to [C_in, C_out], cast bf16
    aw_t = _bf16(np.transpose(np.asarray(attn_w, np.float32), (0, 2, 1)))
    pw_t = _bf16(np.transpose(np.asarray(proj_w, np.float32), (0, 2, 1)))
    fw_t = _bf16(np.transpose(np.asarray(fc_w, np.float32), (0, 2, 1)))
    mw_t = _bf16(np.transpose(np.asarray(mproj_w, np.float32), (0, 2, 1)))
    lm_pad = np.zeros((VPAD, C), np.float32)
    lm_pad[:V] = np.asarray(lm_head_w, np.float32)
    # [VPAD, C] -> [C, VPAD] -> tiles [NVB, 128, KC*512]:
    # line (vb, p) = concat over a of wT[a*128+p, vb*512 : (vb+1)*512]
    lm_t = _bf16(np.ascontiguousarray(
        lm_pad.T.reshape(KC, 128, NVB, 512).transpose(2, 1, 0, 3)
        .reshape(NVB, 128, KC * 512)))

    common = {
        "wte": wte_b,
        "aw": np.ascontiguousarray(aw_t),
        "pw": np.ascontiguousarray(pw_t),
        "fw": np.ascontiguousarray(fw_t),
        "mw": np.ascontiguousarray(mw_t),
        "lmw": lm_t,
        "onesr": _bf16(np.ones((1, 128))),
        "onescr": np.ones((128, 1), np.float32),
        "onescb": _bf16(np.ones((128, 1))),
        "vones": _bf16(np.ones((128, H))),
    }
    if ln_affine:
        def pp(w):  # [C] -> [128, KC]
            return np.ascontiguousarray(np.asarray(w, np.float32)
                                        .reshape(KC, 128).T)
        common["lnw"] = np.stack(
            [pp(w) for l in range(L) for w in (ln1_w[l], ln2_w[l])] + [pp(lnf_w)])
        common["lnb"] = np.stack(
            [pp(b) for l in range(L) for b in (ln1_b[l], ln2_b[l])] + [pp(lnf_b)])
    if attn_bias:
        common["abpp"] = np.ascontiguousarray(np.asarray(attn_b, np.float32)
                                              [:, :2 * C].reshape(L, 12, 128)
                                              .transpose(0, 2, 1))
        common["abrow"] = _bf16(np.asarray(attn_b, np.float32)
                                .reshape(L, 1, 3 * C))
    if proj_bias:
        common["pb"] = np.ascontiguousarray(
            np.asarray(proj_b, np.float32).reshape(L, KC, 128).transpose(0, 2, 1))
    if fc_bias:
        common["fcb"] = np.ascontiguousarray(
            np.asarray(fc_b, np.float32).reshape(L, 4 * KC, 128).transpose(0, 2, 1))
    if mproj_bias:
        common["mb"] = np.ascontiguousarray(
            np.asarray(mproj_b, np.float32).reshape(L, KC, 128).transpose(0, 2, 1))

    in_maps = []
    for c in range(NC):
        m = dict(common)
        m["idx"] = idx_flat[TQ * c:TQ * (c + 1)].reshape(TQ, 1)
        m["wpe"] = np.ascontiguousarray(wpe_b[TQ * (c % 2):TQ * (c % 2) + TQ])
        in_maps.append(m)

    res = run_bass_kernel_spmd(nc, in_maps, list(range(NC)))
    out = np.concatenate(
        [res.results[c]["logits"][:, :V] for c in range(NC)], axis=0)
    return out.reshape(B, T, V).astype(np.float32)


# revision 29
# speedup vs baseline: 1.2044x; 1.2044x over previous
"""BERT (12-layer, C=768, B=4, T=1024, V=30522) forward pass on 8 Trainium2 cores.

Sharding: sequence-parallel over the 4096 tokens (512 tokens/core; core c owns
batch item c//2, sequence half c%2). Attention K/V for the other half of the
sequence is obtained with a pair-wise AllReduce in bf16 (K_partner = K_sum -
K_mine). The LM head is token-local: each core computes its own 512 tokens
against the full (padded) vocab, so no final AllGather is needed.

Matmuls run in bf16 (fp32 PSUM accumulation). The residual stream is fp32
(stored as fp32r so the LN statistic matmuls stream at full PE rate). LN
statistics and softmax normalization are fp32. rstd = exp(-0.5*ln(var+eps))
so LN shares the ACT table set with attention's exp (no table thrash).
Partition broadcasts (softmax 1/denom, LN stats) run on GpSimd, keeping the
tensor engine stream free of scalar-chain stalls.
"""
import math
import sys
from contextlib import ExitStack

for _p in ("/opt/trn_rl_repo",):
    if _p not in sys.path:
        sys.path.insert(0, _p)

import numpy as np

import concourse.bass as bass
import concourse.mybir as mybir
import concourse.tile as tile
from concourse.tile_rust import add_dep_helper
from concourse import bacc
from concourse.bass_utils import run_bass_kernel_spmd
from concourse.masks import make_identity

NC = 8          # cores
L = 12          # layers
C = 768         # model dim
H = 12          # heads
D = 64          # head dim
V = 30522       # vocab
B, T = 4, 1024
TQ = 512        # tokens per core
KC = C // 128   # 6 c-chunks
VPAD = 30720    # vocab padded to 60*512
NVB = VPAD // 512   # 60 vocab blocks
LN_EPS = 1e-5

F32 = mybir.dt.float32
F32R = mybir.dt.float32r
BF16 = mybir.dt.bfloat16
AF = mybir.ActivationFunctionType
ALU = mybir.AluOpType
PAIRS = [[0, 1], [2, 3], [4, 5], [6, 7]]


def _layernorm(nc, scr, tiny, pbc, psp, x_tiles, out_pool, out_tag,
               onescr, onescb, affine, lnw_ap, lnb_ap):
    """LN over the feature (partition) axis of feature-major fp32r x tiles.

    Returns KC bf16 output tiles [128, TQ]. Stats fp32; rstd via
    exp(-0.5*ln(var+eps)) (shares the exp ACT table set). Stat broadcast on
    GpSimd (no PE involvement beyond the two reduction matmul chains).
    """
    sq = []
    for c in range(KC):
        sqt = scr.tile([128, TQ], BF16, tag="sq", name=f"sq{c}")
        # on ACT (Square is in every table set): keeps the x^2 work off the
        # DVE critical path that the residual adds already occupy
        nc.scalar.activation(sqt[:], x_tiles[c][:].bitcast(F32), AF.Square)
        sq.append(sqt)
    s1 = psp.tile([128, TQ], F32, tag="ps")
    for c in range(KC):
        nc.tensor.matmul(s1[:1, :], onescr[:], x_tiles[c][:], start=(c == 0),
                         stop=(c == KC - 1))
    s2 = psp.tile([128, TQ], F32, tag="ps")
    for c in range(KC):
        nc.tensor.matmul(s2[:1, :], onescb[:], sq[c][:], start=(c == 0),
                         stop=(c == KC - 1))
    # f32 scratch slots: 0 = m, 1 = e2+eps then msd, 2 = m^2 then ln, 3 = rstd
    lnt = tiny.tile([1, 4 * TQ], F32, tag="lnt", bufs=2)

    def sl(i):
        return lnt[0:1, i * TQ:(i + 1) * TQ]

    inv = 1.0 / C
    nc.vector.tensor_scalar_mul(sl(0), s1[:1, :], inv)              # m
    nc.vector.tensor_scalar(sl(1), s2[:1, :], inv, LN_EPS,
                            ALU.mult, ALU.add)                      # E[x^2]+eps
    nc.vector.tensor_mul(sl(2), sl(0), sl(0))                       # m^2
    nc.vector.tensor_sub(sl(1), sl(1), sl(2))                       # var+eps
    nc.scalar.activation(sl(2), sl(1), AF.Sqrt)                     # sd
    nc.vector.reciprocal(sl(3), sl(2))                              # rstd
    nc.vector.tensor_mul(sl(1), sl(0), sl(3))                       # m*rstd
    bc = pbc.tile([128, 2 * TQ], F32, tag="bc", bufs=2)
    nc.gpsimd.partition_broadcast(bc[:, 0:TQ], sl(3))               # rstd bcast
    nc.gpsimd.partition_broadcast(bc[:, TQ:2 * TQ], sl(1))          # msd bcast
    out = []
    for c in range(KC):
        t1 = scr.tile([128, TQ], F32, tag="lnt1")
        nc.vector.tensor_mul(t1[:], x_tiles[c][:].bitcast(F32), bc[:, 0:TQ])
        o = out_pool.tile([128, TQ], BF16, tag=out_tag)
        if affine:
            t2 = scr.tile([128, TQ], F32, tag="lnt1")
            nc.vector.tensor_sub(t2[:], t1[:], bc[:, TQ:2 * TQ])
            nc.vector.tensor_scalar(o[:], t2[:], lnw_ap[:, c:c + 1],
                                    lnb_ap[:, c:c + 1], ALU.mult, ALU.add)
        else:
            nc.vector.tensor_sub(o[:], t1[:], bc[:, TQ:2 * TQ])
        out.append(o)
    return out


def build(flags: tuple, n_layers: int = L, do_lm: bool = True) -> bacc.Bacc:
    ln_affine, attn_bias, proj_bias, fc_bias, mproj_bias = flags
    nc = bacc.Bacc("TRN2", target_bir_lowering=False, num_devices=NC)

    # ---- external IO ----
    idx = nc.dram_tensor("idx", [TQ, 1], mybir.dt.int32, kind="ExternalInput")
    wte = nc.dram_tensor("wte", [V, C], BF16, kind="ExternalInput")
    wpe = nc.dram_tensor("wpe", [TQ, C], BF16, kind="ExternalInput")
    aw = nc.dram_tensor("aw", [L, C, 3 * C], BF16, kind="ExternalInput")
    pw = nc.dram_tensor("pw", [L, C, C], BF16, kind="ExternalInput")
    fw = nc.dram_tensor("fw", [L, C, 4 * C], BF16, kind="ExternalInput")
    mw = nc.dram_tensor("mw", [L, 4 * C, C], BF16, kind="ExternalInput")
    lmw = nc.dram_tensor("lmw", [NVB, 128, KC * 512], BF16, kind="ExternalInput")
    onesr_d = nc.dram_tensor("onesr", [1, 128], BF16, kind="ExternalInput")
    onescr_d = nc.dram_tensor("onescr", [128, 1], F32R, kind="ExternalInput")
    onescb_d = nc.dram_tensor("onescb", [128, 1], BF16, kind="ExternalInput")
    vones_d = nc.dram_tensor("vones", [128, H], BF16, kind="ExternalInput")
    if ln_affine:
        lnw_d = nc.dram_tensor("lnw", [2 * L + 1, 128, KC], F32, kind="ExternalInput")
        lnb_d = nc.dram_tensor("lnb", [2 * L + 1, 128, KC], F32, kind="ExternalInput")
    if attn_bias:
        abpp_d = nc.dram_tensor("abpp", [L, 128, 12], F32, kind="ExternalInput")
        abrow_d = nc.dram_tensor("abrow", [L, 1, 3 * C], BF16, kind="ExternalInput")
    if proj_bias:
        pb_d = nc.dram_tensor("pb", [L, 128, KC], F32, kind="ExternalInput")
    if fc_bias:
        fcb_d = nc.dram_tensor("fcb", [L, 128, 4 * KC], F32, kind="ExternalInput")
    if mproj_bias:
        mb_d = nc.dram_tensor("mb", [L, 128, KC], F32, kind="ExternalInput")
    logits = nc.dram_tensor("logits", [TQ, VPAD], F32, kind="ExternalOutput")

    with tile.TileContext(nc) as tc, ExitStack() as stack:
        cst = stack.enter_context(tc.tile_pool(name="cst", bufs=1))
        px = stack.enter_context(tc.tile_pool(name="px", bufs=6))
        tiny = stack.enter_context(tc.tile_pool(name="tiny", bufs=3))
        pbc = stack.enter_context(tc.tile_pool(name="pbc", bufs=2))
        dram = stack.enter_context(tc.tile_pool(name="dram", bufs=2, space="DRAM"))
        # persistent weight pool -> deep cross-phase prefetch
        pw768 = stack.enter_context(tc.tile_pool(name="pw768", bufs=26))
        psp = stack.enter_context(tc.tile_pool(name="psp", bufs=8, space="PSUM"))

        onesr = cst.tile([1, 128], BF16, tag="onesr")
        nc.sync.dma_start(onesr[:], onesr_d[:])
        onescr = cst.tile([128, 1], F32R, tag="onescr")
        nc.sync.dma_start(onescr[:], onescr_d[:])
        onescb = cst.tile([128, 1], BF16, tag="onescb")
        nc.sync.dma_start(onescb[:], onescb_d[:])
        vones = cst.tile([128, H], BF16, tag="vones")
        nc.sync.dma_start(vones[:], vones_d[:])
        ident = cst.tile([128, 128], BF16, tag="ident")
        make_identity(nc, ident[:])
        if ln_affine:
            lnw_sb = cst.tile([128, (2 * L + 1) * KC], F32, tag="lnw")
            nc.sync.dma_start(lnw_sb[:], lnw_d[:].rearrange("a p c -> p (a c)"))
            lnb_sb = cst.tile([128, (2 * L + 1) * KC], F32, tag="lnb")
            nc.sync.dma_start(lnb_sb[:], lnb_d[:].rearrange("a p c -> p (a c)"))
        if attn_bias:
            abpp_sb = cst.tile([128, L * 12], F32, tag="abpp")
            nc.sync.dma_start(abpp_sb[:], abpp_d[:].rearrange("a p c -> p (a c)"))
            abrow_sb = cst.tile([1, L * 3 * C], BF16, tag="abrow")
            nc.sync.dma_start(abrow_sb[:], abrow_d[:].rearrange("a p c -> p (a c)"))
        if proj_bias:
            pb_sb = cst.tile([128, L * KC], F32, tag="pb")
            nc.sync.dma_start(pb_sb[:], pb_d[:].rearrange("a p c -> p (a c)"))
        if fc_bias:
            fcb_sb = cst.tile([128, L * 4 * KC], F32, tag="fcb")
            nc.sync.dma_start(fcb_sb[:], fcb_d[:].rearrange("a p c -> p (a c)"))
        if mproj_bias:
            mb_sb = cst.tile([128, L * KC], F32, tag="mb")
            nc.sync.dma_start(mb_sb[:], mb_d[:].rearrange("a p c -> p (a c)"))

        # residual stream, fp32r, persistent
        x_tiles = [px.tile([128, TQ], F32R, tag="x", name=f"x{i}")
                   for i in range(KC)]

        # ---- embedding: x = wte[idx] + wpe ----
        with tc.tile_pool(name="emb", bufs=5) as emb:
            for tt in range(4):
                it = emb.tile([128, 1], mybir.dt.int32, tag="it")
                nc.sync.dma_start(it[:], idx[128 * tt:128 * (tt + 1), :])
                g = emb.tile([128, C], BF16, tag="g")
                nc.gpsimd.indirect_dma_start(
                    out=g[:], out_offset=None, in_=wte[:],
                    in_offset=bass.IndirectOffsetOnAxis(ap=it[:, :1], axis=0))
                wp = emb.tile([128, C], BF16, tag="wp")
                nc.sync.dma_start(wp[:], wpe[128 * tt:128 * (tt + 1), :])
                xa = emb.tile([128, C], BF16, tag="xa")
                nc.vector.tensor_add(xa[:], g[:], wp[:])
                for cc in range(KC):
                    pt = psp.tile([128, 128], BF16, tag="ps")
                    nc.tensor.transpose(pt[:], xa[:, 128 * cc:128 * (cc + 1)],
                                        ident[:])
                    nc.vector.tensor_copy(
                        x_tiles[cc][:, 128 * tt:128 * (tt + 1)], pt[:])

        # ---- transformer layers ----
        for l in range(n_layers):
            # ===== Scope A: LN1, QKV, KV exchange, attention, proj =====
            with tc.tile_pool(name="sa_scr", bufs=8) as scr, \
                 tc.tile_pool(name="sa_h", bufs=6) as ph, \
                 tc.tile_pool(name="sa_qk", bufs=12) as pqk, \
                 tc.tile_pool(name="sa_kr", bufs=6) as pkr, \
                 tc.tile_pool(name="sa_v", bufs=8) as pv, \
                 tc.tile_pool(name="sa_at", bufs=6) as pat, \
                 tc.tile_pool(name="sa_rs", bufs=4) as prs, \
                 tc.tile_pool(name="sa_yl", bufs=12) as pyl, \
                 tc.tile_pool(name="sa_y", bufs=6) as py:

                h = _layernorm(
                    nc, scr, tiny, pbc, psp, x_tiles, ph, "h", onescr, onescb,
                    ln_affine,
                    lnw_sb[:, 2 * l * KC:(2 * l + 1) * KC] if ln_affine else None,
                    lnb_sb[:, 2 * l * KC:(2 * l + 1) * KC] if ln_affine else None)

                k_loc = dram.tile([C, TQ], BF16, tag="kloc")
                k_sum = dram.tile([C, TQ], BF16, tag="ksum")
                v_loc = dram.tile([TQ, H * (D + 1)], BF16, tag="vloc")
                v_sum = dram.tile([TQ, H * (D + 1)], BF16, tag="vsum")

                # --- V (token-major, exchange kicked first: its AllReduce
                # --- result is needed latest into the remote pass) ---
                wv = []
                for kc in range(KC):
                    wt = pw768.tile([128, C], BF16, tag="w768", name=f"wv{kc}")
                    nc.sync.dma_start(wt[:], aw[l, 128 * kc:128 * (kc + 1),
                                                2 * C:3 * C])
                    wv.append(wt)
                v_t = []
                for tt in range(4):
                    va = psp.tile([128, TQ], F32, tag="ps")
                    vb2 = psp.tile([128, TQ], F32, tag="ps")
                    first = 0
                    if attn_bias:
                        brow = abrow_sb[:, l * 3 * C + 2 * C:l * 3 * C + 3 * C]
                        nc.tensor.matmul(va[:, :512], onesr[:, :128],
                                         brow[:, 0:512], start=True, stop=False)
                        nc.tensor.matmul(vb2[:, :256], onesr[:, :128],
                                         brow[:, 512:768], start=True,
                                         stop=False)
                        first = 1
                    for kc in range(KC):
                        lhs = h[kc][:, 128 * tt:128 * (tt + 1)]
                        nc.tensor.matmul(va[:, :512], lhs, wv[kc][:, 0:512],
                                         start=(kc == 0 and not first),
                                         stop=(kc == KC - 1))
                        nc.tensor.matmul(vb2[:, :256], lhs, wv[kc][:, 512:768],
                                         start=(kc == 0 and not first),
                                         stop=(kc == KC - 1))
                    vt = pv.tile([128, H * (D + 1)], BF16, tag="v",
                                 name=f"vt{tt}")
                    vv = vt[:].rearrange("p (h e) -> p h e", e=D + 1)
                    nc.vector.tensor_copy(
                        vv[:, 0:8, 0:D],
                        va[:, :512].rearrange("p (h e) -> p h e", e=D))
                    nc.vector.tensor_copy(
                        vv[:, 8:12, 0:D],
                        vb2[:, :256].rearrange("p (h e) -> p h e", e=D))
                    nc.vector.tensor_copy(
                        vv[:, :, D:D + 1],
                        vones[:].rearrange("p (h o) -> p h o", o=1))
                    v_t.append(vt)
                    nc.sync.dma_start(v_loc[128 * tt:128 * (tt + 1), :], vt[:])
                nc.gpsimd.collective_compute(
                    "AllReduce", ALU.add, replica_groups=PAIRS,
                    ins=[v_loc.opt()], outs=[v_sum.opt()])

                # --- K (exchange kicked second) ---
                wk = []
                for kc in range(KC):
                    wt = pw768.tile([128, C], BF16, tag="w768", name=f"wk{kc}")
                    nc.sync.dma_start(wt[:], aw[l, 128 * kc:128 * (kc + 1),
                                                C:2 * C])
                    wk.append(wt)
                k_t = []
                for oc in range(KC):
                    psl = psp.tile([128, TQ], F32, tag="ps")
                    for kc in range(KC):
                        nc.tensor.matmul(
                            psl[:], wk[kc][:, 128 * oc:128 * (oc + 1)],
                            h[kc][:], start=(kc == 0), stop=(kc == KC - 1))
                    dst = pqk.tile([128, TQ], BF16, tag="qk", name=f"kt{oc}")
                    if attn_bias:
                        nc.vector.tensor_scalar_add(
                            dst[:], psl[:],
                            abpp_sb[:, l * 12 + KC + oc:l * 12 + KC + oc + 1])
                    else:
                        nc.scalar.copy(dst[:], psl[:])
                    k_t.append(dst)
                    nc.sync.dma_start(k_loc[128 * oc:128 * (oc + 1), :], dst[:])
                nc.gpsimd.collective_compute(
                    "AllReduce", ALU.add, replica_groups=PAIRS,
                    ins=[k_loc.opt()], outs=[k_sum.opt()])

                # --- Q ---
                wq = []
                for kc in range(KC):
                    wt = pw768.tile([128, C], BF16, tag="w768", name=f"wq{kc}")
                    nc.sync.dma_start(wt[:], aw[l, 128 * kc:128 * (kc + 1),
                                                0:C])
                    wq.append(wt)
                q_t = []
                for oc in range(KC):
                    psl = psp.tile([128, TQ], F32, tag="ps")
                    for kc in range(KC):
                        nc.tensor.matmul(
                            psl[:], wq[kc][:, 128 * oc:128 * (oc + 1)],
                            h[kc][:], start=(kc == 0), stop=(kc == KC - 1))
                    dst = pqk.tile([128, TQ], BF16, tag="qk", name=f"qt{oc}")
                    if attn_bias:
                        nc.vector.tensor_scalar_add(
                            dst[:], psl[:],
                            abpp_sb[:, l * 12 + oc:l * 12 + oc + 1])
                    else:
                        nc.scalar.copy(dst[:], psl[:])
                    q_t.append(dst)

                # --- remote K/V: load pair sum, subtract mine (in place) ---
                # prefetch proj weights BEFORE the k_sum/v_sum loads: those
                # loads wait on the AllReduce semaphore and block every DMA
                # issued after them on the sync queue
                wp_ = []
                for kc in range(KC):
                    wt = pw768.tile([128, C], BF16, tag="w768", name=f"wp{kc}")
                    nc.sync.dma_start(wt[:], pw[l, 128 * kc:128 * (kc + 1), :])
                    wp_.append(wt)

                # remote K/V: load pair sum (V first — its exchange finishes
                # first), subtract mine (in place)
                v_r = []
                for tt in range(4):
                    vst = pv.tile([128, H * (D + 1)], BF16, tag="v",
                                  name=f"vs{tt}")
                    nc.sync.dma_start(vst[:],
                                      v_sum[128 * tt:128 * (tt + 1), :])
                    nc.vector.tensor_sub(vst[:], vst[:], v_t[tt][:])
                    v_r.append(vst)
                k_r = []
                for kc in range(KC):
                    kst = pkr.tile([128, TQ], BF16, tag="kr", name=f"ks{kc}")
                    nc.sync.dma_start(kst[:],
                                      k_sum[128 * kc:128 * (kc + 1), :])
                    nc.vector.tensor_sub(kst[:], kst[:], k_t[kc][:])
                    k_r.append(kst)

                # --- attention pass 1: local halves of all 12 heads; each
                # --- head's partial (AV rows + denom row) is evacuated to
                # --- SBUF so the PE never has to wait for the AllReduce
                y_t = [py.tile([128, TQ], BF16, tag="y", name=f"y{i}")
                       for i in range(KC)]
                yloc = []
                scale = 1.0 / math.sqrt(D)
                for hh in range(H):
                    ct, ro = hh // 2, 64 * (hh % 2)
                    yp = psp.tile([128, TQ], F32, tag="ps", name=f"ypl{hh}")
                    ats = []
                    for sc in range(4):
                        ss = psp.tile([128, TQ], F32, tag="ps")
                        nc.tensor.matmul(
                            ss[:], k_t[ct][ro:ro + D, 128 * sc:128 * (sc + 1)],
                            q_t[ct][ro:ro + D, :], start=True, stop=True)
                        at = pat.tile([128, TQ], BF16, tag="at")
                        nc.scalar.activation(at[:], ss[:], AF.Exp, scale=scale)
                        ats.append(at)
                    for sc in range(4):
                        nc.tensor.matmul(
                            yp[:D + 1, :],
                            v_t[sc][:, hh * (D + 1):(hh + 1) * (D + 1)],
                            ats[sc][:], start=(sc == 0), stop=(sc == 3))
                    yl = pyl.tile([D + 1, TQ], BF16, tag="yl", name=f"yl{hh}")
                    nc.vector.tensor_copy(yl[:], yp[:D + 1, :])
                    yloc.append(yl)

                # --- attention pass 2: remote halves + softmax normalize ---
                for hh in range(H):
                    ct, ro = hh // 2, 64 * (hh % 2)
                    yp = psp.tile([128, TQ], F32, tag="ps", name=f"ypr{hh}")
                    ats = []
                    for sc in range(4):
                        ss = psp.tile([128, TQ], F32, tag="ps")
                        nc.tensor.matmul(
                            ss[:], k_r[ct][ro:ro + D, 128 * sc:128 * (sc + 1)],
                            q_t[ct][ro:ro + D, :], start=True, stop=True)
                        at = pat.tile([128, TQ], BF16, tag="at")
                        nc.scalar.activation(at[:], ss[:], AF.Exp, scale=scale)
                        ats.append(at)
                    for sc in range(4):
                        nc.tensor.matmul(
                            yp[:D + 1, :],
                            v_r[sc][:, hh * (D + 1):(hh + 1) * (D + 1)],
                            ats[sc][:], start=(sc == 0), stop=(sc == 3))
                    rec = tiny.tile([1, 2 * TQ], F32, tag="rec", bufs=4)
                    nc.vector.tensor_add(rec[0:1, 0:TQ], yloc[hh][D:D + 1, :],
                                         yp[D:D + 1, :])
                    nc.vector.reciprocal(rec[0:1, TQ:2 * TQ], rec[0:1, 0:TQ])
                    rsb = prs.tile([64, TQ], F32, tag="rsb")
                    nc.gpsimd.partition_broadcast(rsb[:], rec[0:1, TQ:2 * TQ])
                    tsum = scr.tile([64, TQ], F32, tag="tsum", bufs=3)
                    nc.vector.tensor_add(tsum[:], yp[:D, :], yloc[hh][:D, :])
                    nc.vector.tensor_mul(y_t[ct][ro:ro + D, :], tsum[:],
                                         rsb[:])

                # --- proj + residual ---
                for oc in range(KC):
                    psl = psp.tile([128, TQ], F32, tag="ps")
                    for kc in range(KC):
                        nc.tensor.matmul(
                            psl[:], wp_[kc][:, 128 * oc:128 * (oc + 1)],
                            y_t[kc][:], start=(kc == 0), stop=(kc == KC - 1))
                    if proj_bias:
                        nc.vector.scalar_tensor_tensor(
                            x_tiles[oc][:], psl[:],
                            pb_sb[:, l * KC + oc:l * KC + oc + 1],
                            x_tiles[oc][:].bitcast(F32), ALU.add, ALU.add)
                    else:
                        nc.vector.tensor_add(x_tiles[oc][:],
                                             x_tiles[oc][:].bitcast(F32),
                                             psl[:])

            # ===== Scope B: LN2, fc+gelu, mproj =====
            with tc.tile_pool(name="sb_scr", bufs=8) as scr, \
                 tc.tile_pool(name="sb_h", bufs=6) as ph, \
                 tc.tile_pool(name="sb_g", bufs=25) as pg:

                h2 = _layernorm(
                    nc, scr, tiny, pbc, psp, x_tiles, ph, "h", onescr, onescb,
                    ln_affine,
                    lnw_sb[:, (2 * l + 1) * KC:(2 * l + 2) * KC] if ln_affine else None,
                    lnb_sb[:, (2 * l + 1) * KC:(2 * l + 2) * KC] if ln_affine else None)

                g_t = []
                for ob in range(4):
                    wf = []
                    for kc in range(KC):
                        wt = pw768.tile([128, C], BF16, tag="w768",
                                        name=f"wf{kc}")
                        nc.sync.dma_start(
                            wt[:], fw[l, 128 * kc:128 * (kc + 1),
                                      C * ob:C * (ob + 1)])
                        wf.append(wt)
                    for oc in range(KC):
                        psl = psp.tile([128, TQ], F32, tag="ps")
                        for kc in range(KC):
                            nc.tensor.matmul(
                                psl[:], wf[kc][:, 128 * oc:128 * (oc + 1)],
                                h2[kc][:], start=(kc == 0), stop=(kc == KC - 1))
                        gt = pg.tile([128, TQ], BF16, tag="g")
                        ob_oc = ob * KC + oc
                        bias = (fcb_sb[:, l * 4 * KC + ob_oc:
                                       l * 4 * KC + ob_oc + 1]
                                if fc_bias else 0.0)
                        last_gelu = nc.scalar.activation(
                            gt[:], psl[:], AF.Gelu_apprx_tanh, bias=bias)
                        g_t.append(gt)

                # mproj: oc-outer so each accumulator frees right after its
                # chain and the residual add + LN stat prefix for that chunk
                # overlaps the remaining chains
                wm = []
                for k in range(4 * KC):
                    wt = pw768.tile([128, C], BF16, tag="w768", name="wm")
                    nc.sync.dma_start(wt[:], mw[l, 128 * k:128 * (k + 1), :])
                    wm.append(wt)
                for oc in range(KC):
                    psl = psp.tile([128, TQ], F32, tag="ps")
                    for k in range(4 * KC):
                        nc.tensor.matmul(
                            psl[:], wm[k][:, 128 * oc:128 * (oc + 1)],
                            g_t[k][:], start=(k == 0), stop=(k == 4 * KC - 1))
                    if mproj_bias:
                        nc.vector.scalar_tensor_tensor(
                            x_tiles[oc][:], psl[:],
                            mb_sb[:, l * KC + oc:l * KC + oc + 1],
                            x_tiles[oc][:].bitcast(F32), ALU.add, ALU.add)
                    else:
                        nc.vector.tensor_add(x_tiles[oc][:],
                                             x_tiles[oc][:].bitcast(F32),
                                             psl[:])

        # ---- final LN + token-local LM head over the full padded vocab ----
        if do_lm:
            with tc.tile_pool(name="fl_scr", bufs=8) as scr, \
                 tc.tile_pool(name="fl_h", bufs=6) as ph, \
                 tc.tile_pool(name="lm_w", bufs=4) as plw, \
                 tc.tile_pool(name="lm_out", bufs=4) as plo:
                xf = _layernorm(
                    nc, scr, tiny, pbc, psp, x_tiles, ph, "h", onescr, onescb,
                    ln_affine,
                    lnw_sb[:, 2 * L * KC:(2 * L + 1) * KC] if ln_affine else None,
                    lnb_sb[:, 2 * L * KC:(2 * L + 1) * KC] if ln_affine else None)
                for vb in range(NVB):
                    lw = plw.tile([128, KC * 512], BF16, tag="lw")
                    nc.sync.dma_start(lw[:], lmw[vb])
                    for m in range(4):
                        psl = psp.tile([128, TQ], F32, tag="ps")
                        for kc in range(KC):
                            nc.tensor.matmul(
                                psl[:], xf[kc][:, 128 * m:128 * (m + 1)],
                                lw[:, 512 * kc:512 * (kc + 1)],
                                start=(kc == 0), stop=(kc == KC - 1))
                        osb = plo.tile([128, TQ], F32, tag="lo")
                        if m % 2 == 0:
                            nc.scalar.copy(osb[:], psl[:])
                        else:
                            nc.vector.tensor_copy(osb[:], psl[:])
                        nc.sync.dma_start(
                            logits[128 * m:128 * (m + 1),
                                   512 * vb:512 * (vb + 1)], osb[:])

    nc.compile()
    return nc


_CACHE = {}


def _get_nc(flags):
    if flags not in _CACHE:
        _CACHE[flags] = build(flags)
    return _CACHE[flags]


def _bf16(x):
    import ml_dtypes
    return np.asarray(x, dtype=np.float32).astype(ml_dtypes.bfloat16)


def kernel(idx, wte, wpe, ln1_w, ln1_b, attn_w, attn_b, proj_w, proj_b,
           ln2_w, ln2_b, fc_w, fc_b, mproj_w, mproj_b, lnf_w, lnf_b, lm_head_w):
    idx = np.asarray(idx)
    idx_flat = idx.reshape(B * T).astype(np.int32)
    wte_b = _bf16(wte)
    wpe_b = _bf16(np.asarray(wpe, np.float32)[:T])

    ln_affine = not (
        np.all(ln1_w == 1) and np.all(ln1_b == 0) and np.all(ln2_w == 1)
        and np.all(ln2_b == 0) and np.all(lnf_w == 1) and np.all(lnf_b == 0))
    attn_bias = bool(np.any(attn_b != 0))
    proj_bias = bool(np.any(proj_b != 0))
    fc_bias = bool(np.any(fc_b != 0))
    mproj_bias = bool(np.any(mproj_b != 0))
    flags = (ln_affine, attn_bias, proj_bias, fc_bias, mproj_bias)
    nc = _get_nc(flags)

    # host-side layout prep: transpose weights to [C_in, C_out], cast bf16
    aw_t = _bf16(np.transpose(np.asarray(attn_w, np.float32), (0, 2, 1)))
    pw_t = _bf16(np.transpose(np.asarray(proj_w, np.float32), (0, 2, 1)))
    fw_t = _bf16(np.transpose(np.asarray(fc_w, np.float32), (0, 2, 1)))
    mw_t = _bf16(np.transpose(np.asarray(mproj_w, np.float32), (0, 2, 1)))
    lm_pad = np.zeros((VPAD, C), np.float32)
    lm_pad[:V] = np.asarray(lm_head_w, np.float32)
    # [VPAD, C] -> [C, VPAD] -> tiles [NVB, 128, KC*512]:
    # line (vb, p) = concat over a of wT[a*128+p, vb*512 : (vb+1)*512]
    lm_t = _bf16(np.ascontiguousarray(
        lm_pad.T.reshape(KC, 128, NVB, 512).transpose(2, 1, 0, 3)
        .reshape(NVB, 128, KC * 512)))

    common = {
        "wte": wte_b,
        "aw": np.ascontiguousarray(aw_t),
        "pw": np.ascontiguousarray(pw_t),
        "fw": np.ascontiguousarray(fw_t),
        "mw": np.ascontiguousarray(mw_t),
        "lmw": lm_t,
        "onesr": _bf16(np.ones((1, 128))),
        "onescr": np.ones((128, 1), np.float32),
        "onescb": _bf16(np.ones((128, 1))),
        "vones": _bf16(np.ones((128, H))),
    }
    if ln_affine:
        def pp(w):  # [C] -> [128, KC]
            return np.ascontiguousarray(np.asarray(w, np.float32)
                                        .reshape(KC, 128).T)
        common["lnw"] = np.stack(
            [pp(w) for l in range(L) for w in (ln1_w[l], ln2_w[l])] + [pp(lnf_w)])
        common["lnb"] = np.stack(
            [pp(b) for l in range(L) for b in (ln1_b[l], ln2_b[l])] + [pp(lnf_b)])
    if attn_bias:
        common["abpp"] = np.ascontiguousarray(np.asarray(attn_b, np.float32)
                                              [:, :2 * C].reshape(L, 12, 128)
                                              .transpose(0, 2, 1))
        common["abrow"] = _bf16(np.asarray(attn_b, np.float32)
                                .reshape(L, 1, 3 * C))
    if proj_bias:
        common["pb"] = np.ascontiguousarray(
            np.asarray(proj_b, np.float32).reshape(L, KC, 128).transpose(0, 2, 1))
    if fc_bias:
        common["fcb"] = np.ascontiguousarray(
            np.asarray(fc_b, np.float32).reshape(L, 4 * KC, 128).transpose(0, 2, 1))
    if mproj_bias:
        common["mb"] = np.ascontiguousarray(
            np.asarray(mproj_b, np.float32).reshape(L, KC, 128).transpose(0, 2, 1))

    in_maps = []
    for c in range(NC):
        m = dict(common)
        m["idx"] = idx_flat[TQ * c:TQ * (c + 1)].reshape(TQ, 1)
        m["wpe"] = np.ascontiguousarray(wpe_b[TQ * (c % 2):TQ * (c % 2) + TQ])
        in_maps.append(m)

    res = run_bass_kernel_spmd(nc, in_maps, list(range(NC)))
    out = np.concatenate(
        [res.results[c]["logits"][:, :V] for c in range(NC)], axis=0)
    return out.reshape(B, T, V).astype(np.float32)


# revision 35
# speedup vs baseline: 1.2265x; 1.0183x over previous
"""BERT (12-layer, C=768, B=4, T=1024, V=30522) forward pass on 8 Trainium2 cores.

Sharding: sequence-parallel over the 4096 tokens (512 tokens/core; core c owns
batch item c//2, sequence half c%2). Attention K/V for the other half of the
sequence is obtained with a pair-wise AllReduce in bf16 (K_partner = K_sum -
K_mine). The LM head is token-local: each core computes its own 512 tokens
against the full (padded) vocab, so no final AllGather is needed.

Matmuls run in bf16 (fp32 PSUM accumulation). The residual stream is fp32
(stored as fp32r so the LN statistic matmuls stream at full PE rate). LN
statistics and softmax normalization are fp32. rstd = exp(-0.5*ln(var+eps))
so LN shares the ACT table set with attention's exp (no table thrash).
Partition broadcasts (softmax 1/denom, LN stats) run on GpSimd, keeping the
tensor engine stream free of scalar-chain stalls.
"""
import math
import sys
from contextlib import ExitStack

for _p in ("/opt/trn_rl_repo",):
    if _p not in sys.path:
        sys.path.insert(0, _p)

import numpy as np

import concourse.bass as bass
import concourse.mybir as mybir
import concourse.tile as tile
from concourse.tile_rust import add_dep_helper
from concourse import bacc
from concourse.bass_utils import run_bass_kernel_spmd
from concourse.masks import make_identity

NC = 8          # cores
L = 12          # layers
C = 768         # model dim
H = 12          # heads
D = 64          # head dim
V = 30522       # vocab
B, T = 4, 1024
TQ = 512        # tokens per core
KC = C // 128   # 6 c-chunks
VPAD = 30720    # vocab padded to 60*512
NVB = VPAD // 512   # 60 vocab blocks
LN_EPS = 1e-5

F32 = mybir.dt.float32
F32R = mybir.dt.float32r
BF16 = mybir.dt.bfloat16
AF = mybir.ActivationFunctionType
ALU = mybir.AluOpType
PAIRS = [[0, 1], [2, 3], [4, 5], [6, 7]]


def _layernorm(nc, scr, tiny, onesrr, psp, x_tiles, out_pool, out_tag,
               onescr, onescb, affine, lnw_ap, lnb_ap):
    """LN over the feature (partition) axis of feature-major fp32r x tiles.

    Returns KC bf16 output tiles [128, TQ]. Stats fp32; sd via ACT Sqrt
    (table load hides behind the preceding phase), rstd via the fast DVE
    reciprocal. Stat broadcast via two K=1 PE matmuls into one 2-bank PSUM
    tile — the PE is idle during this chain anyway, and it is faster than
    two serial GpSimd partition broadcasts.
    """
    sq = []
    for c in range(KC):
        sqt = scr.tile([128, TQ], BF16, tag="sq", name=f"sq{c}")
        # on ACT (Square is in every table set): keeps the x^2 work off the
        # DVE critical path that the residual adds already occupy
        nc.scalar.activation(sqt[:], x_tiles[c][:].bitcast(F32), AF.Square)
        sq.append(sqt)
    s1 = psp.tile([128, TQ], F32, tag="ps")
    for c in range(KC):
        nc.tensor.matmul(s1[:1, :], onescr[:], x_tiles[c][:], start=(c == 0),
                         stop=(c == KC - 1))
    s2 = psp.tile([128, TQ], F32, tag="ps")
    for c in range(KC):
        nc.tensor.matmul(s2[:1, :], onescb[:], sq[c][:], start=(c == 0),
                         stop=(c == KC - 1))
    # f32 scratch slots: 0 = m, 1 = e2+eps then msd, 2 = m^2 then ln, 3 = rstd
    lnt = tiny.tile([1, 4 * TQ], F32, tag="lnt", bufs=2)

    def sl(i):
        return lnt[0:1, i * TQ:(i + 1) * TQ]

    inv = 1.0 / C
    nc.vector.tensor_scalar_mul(sl(0), s1[:1, :], inv)              # m
    nc.vector.tensor_scalar(sl(1), s2[:1, :], inv, LN_EPS,
                            ALU.mult, ALU.add)                      # E[x^2]+eps
    nc.vector.tensor_mul(sl(2), sl(0), sl(0))                       # m^2
    nc.vector.tensor_sub(sl(1), sl(1), sl(2))                       # var+eps
    nc.scalar.activation(sl(2), sl(1), AF.Sqrt)                     # sd
    lntr = tiny.tile([1, 2 * TQ], F32R, tag="lntr", bufs=2)
    with nc.allow_low_precision(reason="fp32r rounding of LN stats"):
        nc.vector.reciprocal(lntr[0:1, 0:TQ], sl(2))                # rstd
    nc.vector.tensor_mul(lntr[0:1, TQ:2 * TQ], sl(0),
                         lntr[0:1, 0:TQ].bitcast(F32))              # m*rstd
    bc = psp.tile([128, 2 * TQ], F32, tag="ss", bufs=2)
    nc.tensor.matmul(bc[:, 0:TQ], onesrr[:], lntr[0:1, 0:TQ],
                     start=True, stop=True)                         # rstd bcast
    nc.tensor.matmul(bc[:, TQ:2 * TQ], onesrr[:], lntr[0:1, TQ:2 * TQ],
                     start=True, stop=True)                         # msd bcast
    out = []
    for c in range(KC):
        t1 = scr.tile([128, TQ], F32, tag="lnt1")
        nc.vector.tensor_mul(t1[:], x_tiles[c][:].bitcast(F32), bc[:, 0:TQ])
        o = out_pool.tile([128, TQ], BF16, tag=out_tag)
        if affine:
            t2 = scr.tile([128, TQ], F32, tag="lnt1")
            nc.vector.tensor_sub(t2[:], t1[:], bc[:, TQ:2 * TQ])
            nc.vector.tensor_scalar(o[:], t2[:], lnw_ap[:, c:c + 1],
                                    lnb_ap[:, c:c + 1], ALU.mult, ALU.add)
        else:
            nc.vector.tensor_sub(o[:], t1[:], bc[:, TQ:2 * TQ])
        out.append(o)
    return out


def build(flags: tuple, n_layers: int = L, do_lm: bool = True) -> bacc.Bacc:
    ln_affine, attn_bias, proj_bias, fc_bias, mproj_bias = flags
    nc = bacc.Bacc("TRN2", target_bir_lowering=False, num_devices=NC)

    # ---- external IO ----
    idx = nc.dram_tensor("idx", [TQ, 1], mybir.dt.int32, kind="ExternalInput")
    wte = nc.dram_tensor("wte", [V, C], BF16, kind="ExternalInput")
    wpe = nc.dram_tensor("wpe", [TQ, C], BF16, kind="ExternalInput")
    aw = nc.dram_tensor("aw", [L, C, 3 * C], BF16, kind="ExternalInput")
    pw = nc.dram_tensor("pw", [L, C, C], BF16, kind="ExternalInput")
    fw = nc.dram_tensor("fw", [L, C, 4 * C], BF16, kind="ExternalInput")
    mw = nc.dram_tensor("mw", [L, 4 * C, C], BF16, kind="ExternalInput")
    lmw = nc.dram_tensor("lmw", [NVB, 128, KC * 512], BF16, kind="ExternalInput")
    onesr_d = nc.dram_tensor("onesr", [1, 128], BF16, kind="ExternalInput")
    onesrr_d = nc.dram_tensor("onesrr", [1, 128], F32R, kind="ExternalInput")
    onescr_d = nc.dram_tensor("onescr", [128, 1], F32R, kind="ExternalInput")
    onescb_d = nc.dram_tensor("onescb", [128, 1], BF16, kind="ExternalInput")
    vones_d = nc.dram_tensor("vones", [128, H], BF16, kind="ExternalInput")
    if ln_affine:
        lnw_d = nc.dram_tensor("lnw", [2 * L + 1, 128, KC], F32, kind="ExternalInput")
        lnb_d = nc.dram_tensor("lnb", [2 * L + 1, 128, KC], F32, kind="ExternalInput")
    if attn_bias:
        abpp_d = nc.dram_tensor("abpp", [L, 128, 12], F32, kind="ExternalInput")
        abrow_d = nc.dram_tensor("abrow", [L, 1, 3 * C], BF16, kind="ExternalInput")
    if proj_bias:
        pb_d = nc.dram_tensor("pb", [L, 128, KC], F32, kind="ExternalInput")
    if fc_bias:
        fcb_d = nc.dram_tensor("fcb", [L, 128, 4 * KC], F32, kind="ExternalInput")
    if mproj_bias:
        mb_d = nc.dram_tensor("mb", [L, 128, KC], F32, kind="ExternalInput")
    logits = nc.dram_tensor("logits", [TQ, VPAD], F32, kind="ExternalOutput")

    with tile.TileContext(nc) as tc, ExitStack() as stack:
        cst = stack.enter_context(tc.tile_pool(name="cst", bufs=1))
        px = stack.enter_context(tc.tile_pool(name="px", bufs=6))
        tiny = stack.enter_context(tc.tile_pool(name="tiny", bufs=3))
        dram = stack.enter_context(tc.tile_pool(name="dram", bufs=2, space="DRAM"))
        # persistent weight pool -> deep cross-phase prefetch
        pw768 = stack.enter_context(tc.tile_pool(name="pw768", bufs=30))
        psp = stack.enter_context(tc.tile_pool(name="psp", bufs=4, space="PSUM"))

        onesr = cst.tile([1, 128], BF16, tag="onesr")
        nc.sync.dma_start(onesr[:], onesr_d[:])
        onesrr = cst.tile([1, 128], F32R, tag="onesrr")
        nc.sync.dma_start(onesrr[:], onesrr_d[:])
        onescr = cst.tile([128, 1], F32R, tag="onescr")
        nc.sync.dma_start(onescr[:], onescr_d[:])
        onescb = cst.tile([128, 1], BF16, tag="onescb")
        nc.sync.dma_start(onescb[:], onescb_d[:])
        vones = cst.tile([128, H], BF16, tag="vones")
        nc.sync.dma_start(vones[:], vones_d[:])
        ident = cst.tile([128, 128], BF16, tag="ident")
        make_identity(nc, ident[:])
        if ln_affine:
            lnw_sb = cst.tile([128, (2 * L + 1) * KC], F32, tag="lnw")
            nc.sync.dma_start(lnw_sb[:], lnw_d[:].rearrange("a p c -> p (a c)"))
            lnb_sb = cst.tile([128, (2 * L + 1) * KC], F32, tag="lnb")
            nc.sync.dma_start(lnb_sb[:], lnb_d[:].rearrange("a p c -> p (a c)"))
        if attn_bias:
            abpp_sb = cst.tile([128, L * 12], F32, tag="abpp")
            nc.sync.dma_start(abpp_sb[:], abpp_d[:].rearrange("a p c -> p (a c)"))
            abrow_sb = cst.tile([1, L * 3 * C], BF16, tag="abrow")
            nc.sync.dma_start(abrow_sb[:], abrow_d[:].rearrange("a p c -> p (a c)"))
        if proj_bias:
            pb_sb = cst.tile([128, L * KC], F32, tag="pb")
            nc.sync.dma_start(pb_sb[:], pb_d[:].rearrange("a p c -> p (a c)"))
        if fc_bias:
            fcb_sb = cst.tile([128, L * 4 * KC], F32, tag="fcb")
            nc.sync.dma_start(fcb_sb[:], fcb_d[:].rearrange("a p c -> p (a c)"))
        if mproj_bias:
            mb_sb = cst.tile([128, L * KC], F32, tag="mb")
            nc.sync.dma_start(mb_sb[:], mb_d[:].rearrange("a p c -> p (a c)"))

        # residual stream, fp32r, persistent
        x_tiles = [px.tile([128, TQ], F32R, tag="x", name=f"x{i}")
                   for i in range(KC)]

        # ---- embedding: x = wte[idx] + wpe ----
        with tc.tile_pool(name="emb", bufs=5) as emb:
            for tt in range(4):
                it = emb.tile([128, 1], mybir.dt.int32, tag="it")
                nc.sync.dma_start(it[:], idx[128 * tt:128 * (tt + 1), :])
                g = emb.tile([128, C], BF16, tag="g")
                nc.gpsimd.indirect_dma_start(
                    out=g[:], out_offset=None, in_=wte[:],
                    in_offset=bass.IndirectOffsetOnAxis(ap=it[:, :1], axis=0))
                wp = emb.tile([128, C], BF16, tag="wp")
                nc.sync.dma_start(wp[:], wpe[128 * tt:128 * (tt + 1), :])
                xa = emb.tile([128, C], BF16, tag="xa")
                nc.vector.tensor_add(xa[:], g[:], wp[:])
                for cc in range(KC):
                    pt = psp.tile([128, 128], BF16, tag="ps")
                    nc.tensor.transpose(pt[:], xa[:, 128 * cc:128 * (cc + 1)],
                                        ident[:])
                    nc.vector.tensor_copy(
                        x_tiles[cc][:, 128 * tt:128 * (tt + 1)], pt[:])

        # ---- transformer layers ----
        for l in range(n_layers):
            # ===== Scope A: LN1, QKV, KV exchange, attention, proj =====
            with tc.tile_pool(name="sa_scr", bufs=8) as scr, \
                 tc.tile_pool(name="sa_h", bufs=6) as ph, \
                 tc.tile_pool(name="sa_qk", bufs=12) as pqk, \
                 tc.tile_pool(name="sa_kr", bufs=6) as pkr, \
                 tc.tile_pool(name="sa_v", bufs=8) as pv, \
                 tc.tile_pool(name="sa_at", bufs=6) as pat, \
                 tc.tile_pool(name="sa_rs", bufs=4) as prs, \
                 tc.tile_pool(name="sa_yl", bufs=12) as pyl, \
                 tc.tile_pool(name="sa_y", bufs=6) as py:

                h = _layernorm(
                    nc, scr, tiny, onesrr, psp, x_tiles, ph, "h", onescr, onescb,
                    ln_affine,
                    lnw_sb[:, 2 * l * KC:(2 * l + 1) * KC] if ln_affine else None,
                    lnb_sb[:, 2 * l * KC:(2 * l + 1) * KC] if ln_affine else None)

                k_loc = dram.tile([C, TQ], BF16, tag="kloc")
                k_sum = dram.tile([C, TQ], BF16, tag="ksum")
                v_loc = dram.tile([TQ, H * (D + 1)], BF16, tag="vloc")
                v_sum = dram.tile([TQ, H * (D + 1)], BF16, tag="vsum")

                # --- V (token-major, exchange kicked first: its AllReduce
                # --- result is needed latest into the remote pass) ---
                wv = []
                for kc in range(KC):
                    wt = pw768.tile([128, C], BF16, tag="w768", name=f"wv{kc}")
                    nc.sync.dma_start(wt[:], aw[l, 128 * kc:128 * (kc + 1),
                                                2 * C:3 * C])
                    wv.append(wt)
                v_t = []
                for tt in range(4):
                    va = psp.tile([128, TQ], F32, tag="ps")
                    vb2 = psp.tile([128, TQ], F32, tag="ps")
                    first = 0
                    if attn_bias:
                        brow = abrow_sb[:, l * 3 * C + 2 * C:l * 3 * C + 3 * C]
                        nc.tensor.matmul(va[:, :512], onesr[:, :128],
                                         brow[:, 0:512], start=True, stop=False)
                        nc.tensor.matmul(vb2[:, :256], onesr[:, :128],
                                         brow[:, 512:768], start=True,
                                         stop=False)
                        first = 1
                    for kc in range(KC):
                        lhs = h[kc][:, 128 * tt:128 * (tt + 1)]
                        nc.tensor.matmul(va[:, :512], lhs, wv[kc][:, 0:512],
                                         start=(kc == 0 and not first),
                                         stop=(kc == KC - 1))
                        nc.tensor.matmul(vb2[:, :256], lhs, wv[kc][:, 512:768],
                                         start=(kc == 0 and not first),
                                         stop=(kc == KC - 1))
                    vt = pv.tile([128, H * (D + 1)], BF16, tag="v",
                                 name=f"vt{tt}")
                    vv = vt[:].rearrange("p (h e) -> p h e", e=D + 1)
                    nc.vector.tensor_copy(
                        vv[:, 0:8, 0:D],
                        va[:, :512].rearrange("p (h e) -> p h e", e=D))
                    nc.vector.tensor_copy(
                        vv[:, 8:12, 0:D],
                        vb2[:, :256].rearrange("p (h e) -> p h e", e=D))
                    nc.vector.tensor_copy(
                        vv[:, :, D:D + 1],
                        vones[:].rearrange("p (h o) -> p h o", o=1))
                    v_t.append(vt)
                    nc.sync.dma_start(v_loc[128 * tt:128 * (tt + 1), :], vt[:])
                nc.gpsimd.collective_compute(
                    "AllReduce", ALU.add, replica_groups=PAIRS,
                    ins=[v_loc.opt()], outs=[v_sum.opt()])

                # --- K (exchange kicked second) ---
                wk = []
                for kc in range(KC):
                    wt = pw768.tile([128, C], BF16, tag="w768", name=f"wk{kc}")
                    nc.sync.dma_start(wt[:], aw[l, 128 * kc:128 * (kc + 1),
                                                C:2 * C])
                    wk.append(wt)
                k_t = []
                for oc in range(KC):
                    psl = psp.tile([128, TQ], F32, tag="ps")
                    for kc in range(KC):
                        nc.tensor.matmul(
                            psl[:], wk[kc][:, 128 * oc:128 * (oc + 1)],
                            h[kc][:], start=(kc == 0), stop=(kc == KC - 1))
                    dst = pqk.tile([128, TQ], BF16, tag="qk", name=f"kt{oc}")
                    if attn_bias:
                        nc.vector.tensor_scalar_add(
                            dst[:], psl[:],
                            abpp_sb[:, l * 12 + KC + oc:l * 12 + KC + oc + 1])
                    else:
                        nc.scalar.copy(dst[:], psl[:])
                    k_t.append(dst)
                    nc.sync.dma_start(k_loc[128 * oc:128 * (oc + 1), :], dst[:])
                nc.gpsimd.collective_compute(
                    "AllReduce", ALU.add, replica_groups=PAIRS,
                    ins=[k_loc.opt()], outs=[k_sum.opt()])

                # --- Q ---
                wq = []
                for kc in range(KC):
                    wt = pw768.tile([128, C], BF16, tag="w768", name=f"wq{kc}")
                    nc.sync.dma_start(wt[:], aw[l, 128 * kc:128 * (kc + 1),
                                                0:C])
                    wq.append(wt)
                q_t = []
                for oc in range(KC):
                    psl = psp.tile([128, TQ], F32, tag="ps")
                    for kc in range(KC):
                        nc.tensor.matmul(
                            psl[:], wq[kc][:, 128 * oc:128 * (oc + 1)],
                            h[kc][:], start=(kc == 0), stop=(kc == KC - 1))
                    dst = pqk.tile([128, TQ], BF16, tag="qk", name=f"qt{oc}")
                    if attn_bias:
                        nc.vector.tensor_scalar_add(
                            dst[:], psl[:],
                            abpp_sb[:, l * 12 + oc:l * 12 + oc + 1])
                    else:
                        nc.scalar.copy(dst[:], psl[:])
                    q_t.append(dst)

                # --- remote K/V: load pair sum, subtract mine (in place) ---
                # prefetch proj weights BEFORE the k_sum/v_sum loads: those
                # loads wait on the AllReduce semaphore and block every DMA
                # issued after them on the sync queue
                wp_ = []
                for kc in range(KC):
                    wt = pw768.tile([128, C], BF16, tag="w768", name=f"wp{kc}")
                    nc.sync.dma_start(wt[:], pw[l, 128 * kc:128 * (kc + 1), :])
                    wp_.append(wt)

                # remote K/V: load pair sum (V first — its exchange finishes
                # first), subtract mine (in place)
                v_r = []
                for tt in range(4):
                    vst = pv.tile([128, H * (D + 1)], BF16, tag="v",
                                  name=f"vs{tt}")
                    nc.sync.dma_start(vst[:],
                                      v_sum[128 * tt:128 * (tt + 1), :])
                    nc.vector.tensor_sub(vst[:], vst[:], v_t[tt][:])
                    v_r.append(vst)
                k_r = []
                for kc in range(KC):
                    kst = pkr.tile([128, TQ], BF16, tag="kr", name=f"ks{kc}")
                    nc.sync.dma_start(kst[:],
                                      k_sum[128 * kc:128 * (kc + 1), :])
                    nc.vector.tensor_sub(kst[:], kst[:], k_t[kc][:])
                    k_r.append(kst)

                # --- attention pass 1: local halves of all 12 heads; each
                # --- head's partial (AV rows + denom row) is evacuated to
                # --- SBUF so the PE never has to wait for the AllReduce
                y_t = [py.tile([128, TQ], BF16, tag="y", name=f"y{i}")
                       for i in range(KC)]
                yloc = []
                scale = 1.0 / math.sqrt(D)
                for hh in range(H):
                    ct, ro = hh // 2, 64 * (hh % 2)
                    yp = psp.tile([128, TQ], F32, tag="ps", name=f"ypl{hh}")
                    ats = []
                    for sg in range(2):
                        ss = psp.tile([128, 2 * TQ], F32, tag="ss", bufs=2)
                        for j in range(2):
                            sc = 2 * sg + j
                            nc.tensor.matmul(
                                ss[:, TQ * j:TQ * (j + 1)],
                                k_t[ct][ro:ro + D, 128 * sc:128 * (sc + 1)],
                                q_t[ct][ro:ro + D, :], start=True, stop=True)
                        at = pat.tile([128, 2 * TQ], BF16, tag="at")
                        nc.scalar.activation(at[:], ss[:], AF.Exp, scale=scale)
                        ats.append(at)
                    for sc in range(4):
                        nc.tensor.matmul(
                            yp[:D + 1, :],
                            v_t[sc][:, hh * (D + 1):(hh + 1) * (D + 1)],
                            ats[sc // 2][:, TQ * (sc % 2):TQ * (sc % 2 + 1)],
                            start=(sc == 0), stop=(sc == 3))
                    yl = pyl.tile([D + 1, TQ], BF16, tag="yl", name=f"yl{hh}")
                    nc.vector.tensor_copy(yl[:], yp[:D + 1, :])
                    yloc.append(yl)

                # --- attention pass 2: remote halves + softmax normalize ---
                for hh in range(H):
                    ct, ro = hh // 2, 64 * (hh % 2)
                    yp = psp.tile([128, TQ], F32, tag="ps", name=f"ypr{hh}")
                    ats = []
                    for sg in range(2):
                        ss = psp.tile([128, 2 * TQ], F32, tag="ss", bufs=2)
                        for j in range(2):
                            sc = 2 * sg + j
                            nc.tensor.matmul(
                                ss[:, TQ * j:TQ * (j + 1)],
                                k_r[ct][ro:ro + D, 128 * sc:128 * (sc + 1)],
                                q_t[ct][ro:ro + D, :], start=True, stop=True)
                        at = pat.tile([128, 2 * TQ], BF16, tag="at")
                        nc.scalar.activation(at[:], ss[:], AF.Exp, scale=scale)
                        ats.append(at)
                    for sc in range(4):
                        nc.tensor.matmul(
                            yp[:D + 1, :],
                            v_r[sc][:, hh * (D + 1):(hh + 1) * (D + 1)],
                            ats[sc // 2][:, TQ * (sc % 2):TQ * (sc % 2 + 1)],
                            start=(sc == 0), stop=(sc == 3))
                    rec = tiny.tile([1, 2 * TQ], F32, tag="rec", bufs=4)
                    nc.vector.tensor_add(rec[0:1, 0:TQ], yloc[hh][D:D + 1, :],
                                         yp[D:D + 1, :])
                    nc.vector.reciprocal_approx_fast(rec[0:1, TQ:2 * TQ],
                                                     rec[0:1, 0:TQ])
                    rsb = prs.tile([64, TQ], F32, tag="rsb")
                    nc.gpsimd.partition_broadcast(rsb[:], rec[0:1, TQ:2 * TQ])
                    tsum = scr.tile([64, TQ], F32, tag="tsum", bufs=3)
                    nc.vector.tensor_add(tsum[:], yp[:D, :], yloc[hh][:D, :])
                    nc.vector.tensor_mul(y_t[ct][ro:ro + D, :], tsum[:],
                                         rsb[:])

                # --- proj + residual ---
                for oc in range(KC):
                    psl = psp.tile([128, TQ], F32, tag="ps")
                    for kc in range(KC):
                        nc.tensor.matmul(
                            psl[:], wp_[kc][:, 128 * oc:128 * (oc + 1)],
                            y_t[kc][:], start=(kc == 0), stop=(kc == KC - 1))
                    if proj_bias:
                        nc.vector.scalar_tensor_tensor(
                            x_tiles[oc][:], psl[:],
                            pb_sb[:, l * KC + oc:l * KC + oc + 1],
                            x_tiles[oc][:].bitcast(F32), ALU.add, ALU.add)
                    else:
                        nc.vector.tensor_add(x_tiles[oc][:],
                                             x_tiles[oc][:].bitcast(F32),
                                             psl[:])

            # ===== Scope B: LN2, fc+gelu, mproj =====
            with tc.tile_pool(name="sb_scr", bufs=8) as scr, \
                 tc.tile_pool(name="sb_h", bufs=6) as ph, \
                 tc.tile_pool(name="sb_g", bufs=25) as pg:

                h2 = _layernorm(
                    nc, scr, tiny, onesrr, psp, x_tiles, ph, "h", onescr, onescb,
                    ln_affine,
                    lnw_sb[:, (2 * l + 1) * KC:(2 * l + 2) * KC] if ln_affine else None,
                    lnb_sb[:, (2 * l + 1) * KC:(2 * l + 2) * KC] if ln_affine else None)

                g_t = []
                for ob in range(4):
                    wf = []
                    for kc in range(KC):
                        wt = pw768.tile([128, C], BF16, tag="w768",
                                        name=f"wf{kc}")
                        nc.sync.dma_start(
                            wt[:], fw[l, 128 * kc:128 * (kc + 1),
                                      C * ob:C * (ob + 1)])
                        wf.append(wt)
                    for oc in range(KC):
                        psl = psp.tile([128, TQ], F32, tag="ps")
                        for kc in range(KC):
                            nc.tensor.matmul(
                                psl[:], wf[kc][:, 128 * oc:128 * (oc + 1)],
                                h2[kc][:], start=(kc == 0), stop=(kc == KC - 1))
                        gt = pg.tile([128, TQ], BF16, tag="g")
                        ob_oc = ob * KC + oc
                        bias = (fcb_sb[:, l * 4 * KC + ob_oc:
                                       l * 4 * KC + ob_oc + 1]
                                if fc_bias else 0.0)
                        last_gelu = nc.scalar.activation(
                            gt[:], psl[:], AF.Gelu_apprx_tanh, bias=bias)
                        g_t.append(gt)

                # mproj: oc-outer so each accumulator frees right after its
                # chain and the residual add + LN stat prefix for that chunk
                # overlaps the remaining chains
                wm = []
                for k in range(4 * KC):
                    wt = pw768.tile([128, C], BF16, tag="w768", name="wm")
                    nc.sync.dma_start(wt[:], mw[l, 128 * k:128 * (k + 1), :])
                    wm.append(wt)
                for oc in range(KC):
                    psl = psp.tile([128, TQ], F32, tag="ps")
                    for k in range(4 * KC):
                        nc.tensor.matmul(
                            psl[:], wm[k][:, 128 * oc:128 * (oc + 1)],
                            g_t[k][:], start=(k == 0), stop=(k == 4 * KC - 1))
                    if mproj_bias:
                        nc.vector.scalar_tensor_tensor(
                            x_tiles[oc][:], psl[:],
                            mb_sb[:, l * KC + oc:l * KC + oc + 1],
                            x_tiles[oc][:].bitcast(F32), ALU.add, ALU.add)
                    else:
                        nc.vector.tensor_add(x_tiles[oc][:],
                                             x_tiles[oc][:].bitcast(F32),
                                             psl[:])

        # ---- final LN + token-local LM head over the full padded vocab ----
        if do_lm:
            with tc.tile_pool(name="fl_scr", bufs=8) as scr, \
                 tc.tile_pool(name="fl_h", bufs=6) as ph, \
                 tc.tile_pool(name="lm_w", bufs=4) as plw, \
                 tc.tile_pool(name="lm_out", bufs=4) as plo:
                xf = _layernorm(
                    nc, scr, tiny, onesrr, psp, x_tiles, ph, "h", onescr, onescb,
                    ln_affine,
                    lnw_sb[:, 2 * L * KC:(2 * L + 1) * KC] if ln_affine else None,
                    lnb_sb[:, 2 * L * KC:(2 * L + 1) * KC] if ln_affine else None)
                for vb in range(NVB):
                    lw = plw.tile([128, KC * 512], BF16, tag="lw")
                    nc.sync.dma_start(lw[:], lmw[vb])
                    for m in range(4):
                        psl = psp.tile([128, TQ], F32, tag="ps")
                        for kc in range(KC):
                            nc.tensor.matmul(
                                psl[:], xf[kc][:, 128 * m:128 * (m + 1)],
                                lw[:, 512 * kc:512 * (kc + 1)],
                                start=(kc == 0), stop=(kc == KC - 1))
                        osb = plo.tile([128, TQ], F32, tag="lo")
                        if m % 2 == 0:
                            nc.scalar.copy(osb[:], psl[:])
                        else:
                            nc.vector.tensor_copy(osb[:], psl[:])
                        nc.sync.dma_start(
                            logits[128 * m:128 * (m + 1),
                                   512 * vb:512 * (vb + 1)], osb[:])

    nc.compile()
    return nc


_CACHE = {}


def _get_nc(flags):
    if flags not in _CACHE:
        _CACHE[flags] = build(flags)
    return _CACHE[flags]


def _bf16(x):
    import ml_dtypes
    return np.asarray(x, dtype=np.float32).astype(ml_dtypes.bfloat16)


def kernel(idx, wte, wpe, ln1_w, ln1_b, attn_w, attn_b, proj_w, proj_b,
           ln2_w, ln2_b, fc_w, fc_b, mproj_w, mproj_b, lnf_w, lnf_b, lm_head_w):
    idx = np.asarray(idx)
    idx_flat = idx.reshape(B * T).astype(np.int32)
    wte_b = _bf16(wte)
    wpe_b = _bf16(np.asarray(wpe, np.float32)[:T])

    ln_affine = not (
        np.all(ln1_w == 1) and np.all(ln1_b == 0) and np.all(ln2_w == 1)
        and np.all(ln2_b == 0) and np.all(lnf_w == 1) and np.all(lnf_b == 0))
    attn_bias = bool(np.any(attn_b != 0))
    proj_bias = bool(np.any(proj_b != 0))
    fc_bias = bool(np.any(fc_b != 0))
    mproj_bias = bool(np.any(mproj_b != 0))
    flags = (ln_affine, attn_bias, proj_bias, fc_bias, mproj_bias)
    nc = _get_nc(flags)

    # host-side layout prep: transpose weights to [C_in, C_out], cast bf16
    aw_t = _bf16(np.transpose(np.asarray(attn_w, np.float32), (0, 2, 1)))
    pw_t = _bf16(np.transpose(np.asarray(proj_w, np.float32), (0, 2, 1)))
    fw_t = _bf16(np.transpose(np.asarray(fc_w, np.float32), (0, 2, 1)))
    mw_t = _bf16(np.transpose(np.asarray(mproj_w, np.float32), (0, 2, 1)))
    lm_pad = np.zeros((VPAD, C), np.float32)
    lm_pad[:V] = np.asarray(lm_head_w, np.float32)
    # [VPAD, C] -> [C, VPAD] -> tiles [NVB, 128, KC*512]:
    # line (vb, p) = concat over a of wT[a*128+p, vb*512 : (vb+1)*512]
    lm_t = _bf16(np.ascontiguousarray(
        lm_pad.T.reshape(KC, 128, NVB, 512).transpose(2, 1, 0, 3)
        .reshape(NVB, 128, KC * 512)))

    common = {
        "wte": wte_b,
        "aw": np.ascontiguousarray(aw_t),
        "pw": np.ascontiguousarray(pw_t),
        "fw": np.ascontiguousarray(fw_t),
        "mw": np.ascontiguousarray(mw_t),
        "lmw": lm_t,
        "onesr": _bf16(np.ones((1, 128))),
        "onesrr": np.ones((1, 128), np.float32),
        "onescr": np.ones((128, 1), np.float32),
        "onescb": _bf16(np.ones((128, 1))),
        "vones": _bf16(np.ones((128, H))),
    }
    if ln_affine:
        def pp(w):  # [C] -> [128, KC]
            return np.ascontiguousarray(np.asarray(w, np.float32)
                                        .reshape(KC, 128).T)
        common["lnw"] = np.stack(
            [pp(w) for l in range(L) for w in (ln1_w[l], ln2_w[l])] + [pp(lnf_w)])
        common["lnb"] = np.stack(
            [pp(b) for l in range(L) for b in (ln1_b[l], ln2_b[l])] + [pp(lnf_b)])
    if attn_bias:
        common["abpp"] = np.ascontiguousarray(np.asarray(attn_b, np.float32)
                                              [:, :2 * C].reshape(L, 12, 128)
                                              .transpose(0, 2, 1))
        common["abrow"] = _bf16(np.asarray(attn_b, np.float32)
                                .reshape(L, 1, 3 * C))
    if proj_bias:
        common["pb"] = np.ascontiguousarray(
            np.asarray(proj_b, np.float32).reshape(L, KC, 128).transpose(0, 2, 1))
    if fc_bias:
        common["fcb"] = np.ascontiguousarray(
            np.asarray(fc_b, np.float32).reshape(L, 4 * KC, 128).transpose(0, 2, 1))
    if mproj_bias:
        common["mb"] = np.ascontiguousarray(
            np.asarray(mproj_b, np.float32).reshape(L, KC, 128).transpose(0, 2, 1))

    in_maps = []
    for c in range(NC):
        m = dict(common)
        m["idx"] = idx_flat[TQ * c:TQ * (c + 1)].reshape(TQ, 1)
        m["wpe"] = np.ascontiguousarray(wpe_b[TQ * (c % 2):TQ * (c % 2) + TQ])
        in_maps.append(m)

    res = run_bass_kernel_spmd(nc, in_maps, list(range(NC)))
    out = np.concatenate(
        [res.results[c]["logits"][:, :V] for c in range(NC)], axis=0)
    return out.reshape(B, T, V).astype(np.float32)
